# revision 19
# baseline (speedup 1.0000x reference)
"""Trainium2 Bass kernel for the segment_reduce ArtifactModel problem.

Distribution: 8 NeuronCores, data-parallel by variants. Core c owns variants
[c*4096, (c+1)*4096): their ref segments plus alt segments (B + range). All
segments are core-local, so there is no cross-core reduction.

Per-core device pipeline (feature-major, reads pair-packed):
  xt  [128, S]  partition p = 32*strip + 16*parity + feat, free = pair-col
  GEMM1: K=32 row-tiles (4 strips) with block-diag [w0;b0]    -> psum [128,*]
  relu evac (ACT/DVE alternating)                             -> h1 bf16
  GEMM2: K=128 block-diag w1                                  -> psum [128,*]
  sigmoid (+b1 via ACT bias)                                  -> phi bf16
  segment sums: strided tensor_reduce over PAD/2 pair-cols    -> raw f32
  means = (top+bot of raw  - v1*padcount) / count             (matmul + 2 TT)
  omega/rho/cal MLPs feature-major, tanh epilogue, DMA out [1, 4096]
"""

import math
import numpy as np
import ml_dtypes

BF16 = ml_dtypes.bfloat16

B = 32768
N_CORES = 8
VPC = B // N_CORES           # 4096 variants per core
SEGS = 2 * VPC               # 8192 segments per core
NSTRIP = 4
SEGS_PER_STRIP = SEGS // NSTRIP   # 2048


def _build_graph(PAD, scal, relu_act_ratio=1):
    """Build the SPMD single-core graph.

    scal: dict of python-float scalars baked into instructions
          (br2, bc2, max_logit).
    relu_act_ratio: of every 3 relu evacs, how many go to ACT vs DVE
          (2 -> ACT:DVE = 2:1... actually evac i uses ACT if i % 3 < ratio).
    """
    import concourse.mybir as mybir
    from concourse import bacc
    from concourse.tile import TileContext
    from contextlib import ExitStack

    f32 = mybir.dt.float32
    bf16 = mybir.dt.bfloat16
    AF = mybir.ActivationFunctionType
    ALU = mybir.AluOpType

    PPS = PAD // 2                      # pair-cols per segment
    STRIP_COLS = SEGS_PER_STRIP * PPS   # 32768 for PAD=32
    SUPER = 64 * PPS                    # pair-cols per super-tile (64 segs)
    DMA_COLS = 4096
    assert STRIP_COLS % DMA_COLS == 0 and DMA_COLS % SUPER == 0
    U_PER_DMA = DMA_COLS // SUPER

    nc = bacc.Bacc()

    xt_d = nc.dram_tensor("xt", [128, STRIP_COLS], bf16, kind="ExternalInput")
    wph1_d = nc.dram_tensor("wph1", [128, 256], bf16, kind="ExternalInput")
    cst_d = nc.dram_tensor("cst", [128, 704], f32, kind="ExternalInput")
    cc_d = nc.dram_tensor("cc", [128, SEGS], f32, kind="ExternalInput")  # rows 0:64 c1 (1/c), 64:128 c2 (v1*padn)
    sqc_d = nc.dram_tensor("sqc", [2, VPC], f32, kind="ExternalInput")
    xi_d = nc.dram_tensor("xi", [16, VPC], f32, kind="ExternalInput")
    out_d = nc.dram_tensor("out", [1, VPC], f32, kind="ExternalOutput")

    CO = {}
    off = 0
    for name, w in [("b1t", 1), ("ic2", 64), ("br0", 1), ("br1", 1),
                    ("bo1", 1), ("wc0", 5), ("wc1", 5), ("wc2", 1),
                    ("bc0", 1), ("bc1", 1), ("wr0r", 128), ("wr0a", 128),
                    ("wr0o", 128), ("wr1", 64), ("wr2", 1), ("wo0p", 64),
                    ("wo1", 64)]:
        CO[name] = (off, w)
        off += w
    assert off <= 704

    with TileContext(nc) as tc, ExitStack() as ctx:
        consts = ctx.enter_context(tc.tile_pool(name="consts", bufs=1))
        from contextlib import ExitStack as _ES
        p1_ctx = _ES()
        xt_pool = p1_ctx.enter_context(tc.tile_pool(name="xtp", bufs=2))
        h1_pool = p1_ctx.enter_context(tc.tile_pool(name="h1p", bufs=3))
        phi_pool = p1_ctx.enter_context(tc.tile_pool(name="phip", bufs=3))
        g1_pool = p1_ctx.enter_context(tc.tile_pool(name="g1p", bufs=2, space="PSUM"))
        g2_pool = p1_ctx.enter_context(tc.tile_pool(name="g2p", bufs=2, space="PSUM"))

        # ---- constants ----
        cst_t = consts.tile([128, 704], f32)
        nc.sync.dma_start(out=cst_t[:], in_=cst_d[:, :])
        wph1_t = consts.tile([128, 256], bf16)
        nc.sync.dma_start(out=wph1_t[:], in_=wph1_d[:, :])
        cc_t = consts.tile([128, SEGS], f32)
        nc.sync.dma_start(out=cc_t[:], in_=cc_d[:, :])
        sqc_t = consts.tile([2, VPC], f32)
        nc.sync.dma_start(out=sqc_t[:2, :], in_=sqc_d[:, :])
        xi_t = consts.tile([16, VPC], f32)
        nc.sync.dma_start(out=xi_t[:16, :], in_=xi_d[:, :])

        def C(name, prows=128):
            o, w = CO[name]
            return cst_t[:prows, o:o + w]

        w0p4 = wph1_t[:, 0:128]
        w1bd = wph1_t[:, 128:256]
        wr0r_w = C("wr0r", 64)
        wr0a_w = C("wr0a", 64)
        wr0o_w = C("wr0o", 64)
        wr1_w = C("wr1", 128)
        wr2_w = C("wr2", 64)
        wo0p_w = C("wo0p", 16)
        wo1_w = C("wo1", 64)

        raw_sb = consts.tile([128, SEGS], f32, tag="raw")
        means_sb = consts.tile([64, SEGS], f32, tag="means")
        omega_sb = consts.tile([64, VPC], f32, tag="omega")
        fin_sb = consts.tile([1, VPC], f32, tag="fin")

        # ---- phase 1: phi MLP + segment sums ----
        n_evac = 0
        for k in range(STRIP_COLS // DMA_COLS):
            xt_t = xt_pool.tile([128, DMA_COLS], bf16)
            nc.sync.dma_start(out=xt_t[:], in_=xt_d[:, k * DMA_COLS:(k + 1) * DMA_COLS])
            for u in range(U_PER_DMA):
                for half in range(2):
                    g1s, h1s = [], []
                    for si in range(2):
                        i = 2 * half + si
                        g1 = g1_pool.tile([128, SUPER], f32)
                        rhs = xt_t[32 * i:32 * i + 32, u * SUPER:(u + 1) * SUPER]
                        for q in range(SUPER // 512):
                            nc.tensor.matmul(
                                out=g1[:, q * 512:(q + 1) * 512],
                                lhsT=w0p4[32 * i:32 * i + 32, :],
                                rhs=rhs[:, q * 512:(q + 1) * 512],
                                start=True, stop=True,
                                tile_position=(32 * i, 0))
                        g1s.append(g1)
                    for si in range(2):
                        h1 = h1_pool.tile([128, SUPER], bf16)
                        if n_evac % 3 < relu_act_ratio:
                            nc.scalar.activation(out=h1[:], in_=g1s[si][:], func=AF.Relu)
                        else:
                            nc.vector.tensor_scalar_max(out=h1[:], in0=g1s[si][:], scalar1=0.0)
                        n_evac += 1
                        h1s.append(h1)
                    for si in range(2):
                        i = 2 * half + si
                        g2 = g2_pool.tile([128, SUPER], f32)
                        for q in range(SUPER // 512):
                            nc.tensor.matmul(
                                out=g2[:, q * 512:(q + 1) * 512],
                                lhsT=w1bd[:, :],
                                rhs=h1s[si][:, q * 512:(q + 1) * 512],
                                start=True, stop=True)
                        phi = phi_pool.tile([128, SUPER], bf16)
                        nc.scalar.activation(out=phi[:], in_=g2[:], func=AF.Sigmoid,
                                             bias=C("b1t"))
                        seg0 = i * SEGS_PER_STRIP + (k * U_PER_DMA + u) * 64
                        nc.vector.tensor_reduce(
                            out=raw_sb[:, seg0:seg0 + 64],
                            in_=phi[:].rearrange("p (s j) -> p s j", j=PPS),
                            axis=mybir.AxisListType.X, op=ALU.add)

        p1_ctx.close()  # free phase-1 SBUF/PSUM pools
        mp_pool = ctx.enter_context(tc.tile_pool(name="mpp", bufs=2, space="PSUM"))
        vh_pool = ctx.enter_context(tc.tile_pool(name="vhp", bufs=2))

        # ---- segment means ----
        for s in range(SEGS // 512):
            sl = slice(s * 512, (s + 1) * 512)
            mp = mp_pool.tile([64, 512], f32, tag="mp64")
            nc.tensor.matmul(out=mp[:], lhsT=C("ic2"), rhs=raw_sb[:, sl],
                             start=True, stop=True)
            mt = vh_pool.tile([64, 512], f32, tag="mt")
            nc.vector.tensor_tensor(out=mt[:64], in0=mp[:], in1=cc_t[64:128, sl],
                                    op=ALU.subtract)
            nc.vector.tensor_tensor(out=means_sb[:64, sl], in0=mt[:64],
                                    in1=cc_t[0:64, sl], op=ALU.mult)

        # ---- omega MLP ----
        for s in range(VPC // 512):
            sl = slice(s * 512, (s + 1) * 512)
            p1 = mp_pool.tile([64, 512], f32, tag="mp64")
            nc.tensor.matmul(out=p1[:], lhsT=wo0p_w, rhs=xi_t[:16, sl],
                             start=True, stop=True)
            h = vh_pool.tile([64, 512], f32, tag="vh64")
            nc.scalar.activation(out=h[:64], in_=p1[:], func=AF.Relu)
            p2 = mp_pool.tile([64, 512], f32, tag="mp64")
            nc.tensor.matmul(out=p2[:], lhsT=wo1_w, rhs=h[:64], start=True, stop=True)
            nc.scalar.activation(out=omega_sb[:64, sl], in_=p2[:], func=AF.Sigmoid,
                                 bias=C("bo1", 64))

        # ---- rho + cal + epilogue, chunk-wise ----
        for s in range(VPC // 512):
            sl = slice(s * 512, (s + 1) * 512)
            rp = mp_pool.tile([128, 512], f32, tag="rp128")
            nc.tensor.matmul(out=rp[:], lhsT=wr0r_w, rhs=means_sb[:64, sl],
                             start=True, stop=False)
            nc.tensor.matmul(out=rp[:], lhsT=wr0a_w,
                             rhs=means_sb[:64, VPC + s * 512:VPC + (s + 1) * 512],
                             start=False, stop=False)
            nc.tensor.matmul(out=rp[:], lhsT=wr0o_w, rhs=omega_sb[:64, sl],
                             start=False, stop=True)
            h = vh_pool.tile([128, 512], f32, tag="vh128")
            nc.scalar.activation(out=h[:], in_=rp[:], func=AF.Relu, bias=C("br0"))
            rp2 = mp_pool.tile([64, 512], f32, tag="mp64")
            nc.tensor.matmul(out=rp2[:], lhsT=wr1_w, rhs=h[:], start=True, stop=True)
            h2 = vh_pool.tile([64, 512], f32, tag="vh64")
            nc.scalar.activation(out=h2[:64], in_=rp2[:], func=AF.Relu, bias=C("br1", 64))
            lp = mp_pool.tile([1, 512], f32, tag="mp1")
            nc.tensor.matmul(out=lp[:], lhsT=wr2_w, rhs=h2[:64], start=True, stop=True)
            lg = vh_pool.tile([1, 512], f32, tag="lg")
            nc.vector.tensor_scalar_add(out=lg[:1], in0=lp[:], scalar1=scal["br2"])

            # cal
            cp = mp_pool.tile([5, 512], f32, tag="mp5")
            nc.tensor.matmul(out=cp[:], lhsT=C("wc0", 2), rhs=sqc_t[:2, sl],
                             start=True, stop=True)
            ch = vh_pool.tile([5, 512], f32, tag="vh5")
            nc.scalar.activation(out=ch[:5], in_=cp[:], func=AF.Relu, bias=C("bc0", 5))
            cp2 = mp_pool.tile([5, 512], f32, tag="mp5")
            nc.tensor.matmul(out=cp2[:], lhsT=C("wc1", 5), rhs=ch[:5], start=True, stop=True)
            ch2 = vh_pool.tile([5, 512], f32, tag="vh5")
            nc.scalar.activation(out=ch2[:5], in_=cp2[:], func=AF.Relu, bias=C("bc1", 5))
            cp3 = mp_pool.tile([1, 512], f32, tag="mp1")
            nc.tensor.matmul(out=cp3[:], lhsT=C("wc2", 5), rhs=ch2[:5], start=True, stop=True)
            tp = vh_pool.tile([1, 512], f32, tag="tp")
            nc.vector.tensor_scalar_add(out=tp[:1], in0=cp3[:], scalar1=scal["bc2"])

            # epilogue: fin = ml * tanh(lg*tp/ml)
            ct = vh_pool.tile([1, 512], f32, tag="ct")
            nc.vector.tensor_tensor(out=ct[:1], in0=lg[:1], in1=tp[:1], op=ALU.mult)
            tt = vh_pool.tile([1, 512], f32, tag="tt")
            nc.scalar.activation(out=tt[:1], in_=ct[:1], func=AF.Tanh,
                                 scale=1.0 / scal["ml"])
            nc.vector.tensor_scalar_mul(out=fin_sb[:1, sl], in0=tt[:1],
                                        scalar1=scal["ml"])

        nc.sync.dma_start(out=out_d[:, :], in_=fin_sb[:1, :])

    nc.compile()
    return nc, CO


def _pack_consts(CO, arrs):
    blob = np.zeros((128, 704), np.float32)
    for name, a in arrs.items():
        o, w = CO[name]
        a = np.asarray(a, np.float32)
        blob[:a.shape[0], o:o + w] = a
    return blob


def kernel(**inputs):
    from concourse.bass_utils import run_bass_kernel_spmd

    reads = np.asarray(inputs["reads"], np.float32)
    info = np.asarray(inputs["info"], np.float32)
    end = np.asarray(inputs["end_indices"], np.int64)
    ml = float(np.asarray(inputs["max_logit"]))

    starts = np.concatenate([[0], end[:-1]])
    counts = (end - starts).astype(np.int64)
    assert counts.min() >= 1, "empty segments unsupported"
    PAD = max(32, 2 * math.ceil(counts.max() / 2))

    w0 = np.asarray(inputs["phi_w0"], np.float32); b0 = np.asarray(inputs["phi_b0"], np.float32)
    w1 = np.asarray(inputs["phi_w1"], np.float32); b1 = np.asarray(inputs["phi_b1"], np.float32)

    w0p = np.zeros((32, 128), np.float32)
    for a in range(2):
        w0p[16 * a:16 * a + 11, 64 * a:64 * a + 64] = w0
        w0p[16 * a + 11, 64 * a:64 * a + 64] = b0
    w0p4 = np.zeros((128, 128), np.float32)
    for i in range(NSTRIP):
        w0p4[32 * i:32 * i + 32, :] = w0p
    w1bd = np.zeros((128, 128), np.float32)
    for a in range(2):
        w1bd[64 * a:64 * a + 64, 64 * a:64 * a + 64] = w1

    def pad128(a, cols):
        out = np.zeros((128, cols), np.float32)
        a = np.asarray(a, np.float32)
        out[:a.shape[0], :a.shape[1]] = a
        return out

    wr0 = np.asarray(inputs["rho_w0"], np.float32)
    wo0p_np = np.vstack([np.asarray(inputs["omega_w0"], np.float32),
                         np.asarray(inputs["omega_b0"], np.float32)[None, :],
                         np.zeros((6, 64), np.float32)])
    wph1 = np.concatenate([w0p4, w1bd], axis=1).astype(BF16)

    v1 = (1.0 / (1.0 + np.exp(-b1))).astype(BF16).astype(np.float32)

    scal = {
        "br2": float(np.asarray(inputs["rho_b2"]).reshape(-1)[0]),
        "bc2": float(np.asarray(inputs["cal_b2"]).reshape(-1)[0]),
        "ml": ml,
    }
    nc, CO = _build_graph(PAD, scal)

    cst = _pack_consts(CO, {
        "b1t": np.tile(b1, 2)[:, None],
        "ic2": np.vstack([np.eye(64, dtype=np.float32)] * 2),
        "br0": np.asarray(inputs["rho_b0"], np.float32)[:, None],
        "br1": np.asarray(inputs["rho_b1"], np.float32)[:, None],
        "bo1": np.asarray(inputs["omega_b1"], np.float32)[:, None],
        "wc0": np.asarray(inputs["cal_w0"], np.float32),
        "wc1": np.asarray(inputs["cal_w1"], np.float32),
        "wc2": np.asarray(inputs["cal_w2"], np.float32),
        "bc0": np.asarray(inputs["cal_b0"], np.float32)[:, None],
        "bc1": np.asarray(inputs["cal_b1"], np.float32)[:, None],
        "wr0r": wr0[0:64],
        "wr0a": wr0[64:128],
        "wr0o": wr0[128:192],
        "wr1": np.asarray(inputs["rho_w1"], np.float32),
        "wr2": np.asarray(inputs["rho_w2"], np.float32),
        "wo0p": wo0p_np,
        "wo1": np.asarray(inputs["omega_w1"], np.float32),
    })

    PPS = PAD // 2
    jj = np.arange(PAD)[None, :]
    in_maps = []
    for c in range(N_CORES):
        rid = np.arange(c * VPC, (c + 1) * VPC)
        seg_ids = np.concatenate([rid, B + rid])
        cs = counts[seg_ids]
        ss = starts[seg_ids]
        valid = jj < cs[:, None]
        gidx = np.where(valid, ss[:, None] + jj, 0)
        pr = reads[gidx] * valid[:, :, None]
        feats = np.zeros((SEGS, PAD, 16), np.float32)
        feats[:, :, :11] = pr
        feats[:, :, 11] = valid
        arr = feats.reshape(NSTRIP, SEGS_PER_STRIP, PPS, 2, 16)
        xt = arr.transpose(0, 3, 4, 1, 2).reshape(128, SEGS_PER_STRIP * PPS)

        rc = (1.0 / cs).astype(np.float32)
        padn = (PAD - cs).astype(np.float32)
        cc = np.empty((128, SEGS), np.float32)
        cc[0:64] = np.broadcast_to(rc[None, :], (64, SEGS))
        cc[64:128] = v1[:, None] * padn[None, :]
        in_maps.append({
            "xt": np.ascontiguousarray(xt.astype(BF16)),
            "wph1": wph1,
            "cst": cst,
            "cc": cc,
            "sqc": np.stack([np.sqrt(cs[VPC:].astype(np.float32)),
                             np.sqrt(cs[:VPC].astype(np.float32))]),
            "xi": np.vstack([info[rid].T.astype(np.float32),
                             np.ones((1, VPC), np.float32),
                             np.zeros((6, VPC), np.float32)]),
        })

    res = run_bass_kernel_spmd(nc, in_maps, core_ids=list(range(N_CORES)))
    out = np.concatenate([res.results[c]["out"][0] for c in range(N_CORES)])
    return out.astype(np.float32)


# revision 22
# speedup vs baseline: 1.1545x; 1.1545x over previous
"""Trainium2 Bass kernel for the segment_reduce ArtifactModel problem.

Distribution: 8 NeuronCores, data-parallel by variants. Core c owns variants
[c*4096, (c+1)*4096): their ref segments plus alt segments (B + range). All
segments are core-local, so there is no cross-core reduction.

Per-core device pipeline (feature-major, reads pair-packed):
  xt  [128, S]  partition p = 32*strip + 16*parity + feat, free = pair-col
  GEMM1: K=32 row-tiles (4 strips) with block-diag [w0;b0]    -> psum [128,*]
  relu evac (ACT/DVE alternating)                             -> h1 bf16
  GEMM2: K=128 block-diag w1                                  -> psum [128,*]
  sigmoid (+b1 via ACT bias)                                  -> phi bf16
  segment sums: strided tensor_reduce over PAD/2 pair-cols    -> raw f32
  means = (top+bot of raw  - v1*padcount) / count             (matmul + 2 TT)
  omega/rho/cal MLPs feature-major, tanh epilogue, DMA out [1, 4096]
"""

import math
import numpy as np
import ml_dtypes

BF16 = ml_dtypes.bfloat16

B = 32768
N_CORES = 8
VPC = B // N_CORES           # 4096 variants per core
SEGS = 2 * VPC               # 8192 segments per core
NSTRIP = 4
SEGS_PER_STRIP = SEGS // NSTRIP   # 2048


def _build_graph(PAD, scal, relu_act_ratio=2):
    """Build the SPMD single-core graph.

    scal: dict of python-float scalars baked into instructions
          (br2, bc2, max_logit).
    relu_act_ratio: of every 3 relu evacs, how many go to ACT vs DVE
          (2 -> ACT:DVE = 2:1... actually evac i uses ACT if i % 3 < ratio).
    """
    import concourse.mybir as mybir
    from concourse import bacc
    from concourse.tile import TileContext
    from contextlib import ExitStack

    f32 = mybir.dt.float32
    bf16 = mybir.dt.bfloat16
    AF = mybir.ActivationFunctionType
    ALU = mybir.AluOpType

    PPS = PAD // 2                      # pair-cols per segment
    STRIP_COLS = SEGS_PER_STRIP * PPS   # 32768 for PAD=32
    SUPER = 64 * PPS                    # pair-cols per super-tile (64 segs)
    DMA_COLS = 4096
    assert STRIP_COLS % DMA_COLS == 0 and DMA_COLS % SUPER == 0
    U_PER_DMA = DMA_COLS // SUPER

    nc = bacc.Bacc()

    xt_d = nc.dram_tensor("xt", [128, STRIP_COLS], bf16, kind="ExternalInput")
    wph1_d = nc.dram_tensor("wph1", [128, 256], bf16, kind="ExternalInput")
    cst_d = nc.dram_tensor("cst", [128, 704], f32, kind="ExternalInput")
    cc_d = nc.dram_tensor("cc", [128, SEGS], f32, kind="ExternalInput")  # rows 0:64 c1 (1/c), 64:128 c2 (v1*padn)
    sqc_d = nc.dram_tensor("sqc", [2, VPC], f32, kind="ExternalInput")
    xi_d = nc.dram_tensor("xi", [16, VPC], f32, kind="ExternalInput")
    out_d = nc.dram_tensor("out", [1, VPC], f32, kind="ExternalOutput")

    CO = {}
    off = 0
    for name, w in [("b1t", 1), ("ic2", 64), ("br0", 1), ("br1", 1),
                    ("bo1", 1), ("wc0", 5), ("wc1", 5), ("wc2", 1),
                    ("bc0", 1), ("bc1", 1), ("wr0r", 128), ("wr0a", 128),
                    ("wr0o", 128), ("wr1", 64), ("wr2", 1), ("wo0p", 64),
                    ("wo1", 64)]:
        CO[name] = (off, w)
        off += w
    assert off <= 704

    with TileContext(nc) as tc, ExitStack() as ctx:
        consts = ctx.enter_context(tc.tile_pool(name="consts", bufs=1))
        from contextlib import ExitStack as _ES
        p1_ctx = _ES()
        xt_pool = p1_ctx.enter_context(tc.tile_pool(name="xtp", bufs=2))
        h1_pool = p1_ctx.enter_context(tc.tile_pool(name="h1p", bufs=4))
        phi_pool = p1_ctx.enter_context(tc.tile_pool(name="phip", bufs=4))
        g1_pool = p1_ctx.enter_context(tc.tile_pool(name="g1p", bufs=2, space="PSUM"))
        g2_pool = p1_ctx.enter_context(tc.tile_pool(name="g2p", bufs=2, space="PSUM"))

        # ---- constants ----
        cst_t = consts.tile([128, 704], f32)
        nc.sync.dma_start(out=cst_t[:], in_=cst_d[:, :])
        wph1_t = consts.tile([128, 256], bf16)
        nc.sync.dma_start(out=wph1_t[:], in_=wph1_d[:, :])
        cc_t = consts.tile([128, SEGS], f32)
        nc.sync.dma_start(out=cc_t[:], in_=cc_d[:, :])
        sqc_t = consts.tile([2, VPC], f32)
        nc.sync.dma_start(out=sqc_t[:2, :], in_=sqc_d[:, :])
        xi_t = consts.tile([16, VPC], f32)
        nc.sync.dma_start(out=xi_t[:16, :], in_=xi_d[:, :])

        def C(name, prows=128):
            o, w = CO[name]
            return cst_t[:prows, o:o + w]

        w0p4 = wph1_t[:, 0:128]
        w1bd = wph1_t[:, 128:256]
        wr0r_w = C("wr0r", 64)
        wr0a_w = C("wr0a", 64)
        wr0o_w = C("wr0o", 64)
        wr1_w = C("wr1", 128)
        wr2_w = C("wr2", 64)
        wo0p_w = C("wo0p", 16)
        wo1_w = C("wo1", 64)

        raw_sb = consts.tile([128, SEGS], f32, tag="raw")
        means_sb = consts.tile([64, SEGS], f32, tag="means")
        omega_sb = consts.tile([64, VPC], f32, tag="omega")
        fin_sb = consts.tile([1, VPC], f32, tag="fin")

        # ---- PE warmup: sustained MM burst to flip HAM to 8/8 ----
        warm = g1_pool.tile([128, SUPER], f32, tag="g1")
        for w in range(28):
            nc.tensor.matmul(out=warm[:, 0:256], lhsT=w1bd[:, :],
                             rhs=wph1_t[:, 0:256], start=True, stop=True)

        # ---- phase 1: phi MLP + segment sums ----
        n_evac = 0
        for k in range(STRIP_COLS // DMA_COLS):
            xt_t = xt_pool.tile([128, DMA_COLS], bf16)
            nc.sync.dma_start(out=xt_t[:], in_=xt_d[:, k * DMA_COLS:(k + 1) * DMA_COLS])
            for u in range(U_PER_DMA):
                for half in range(2):
                    g1s, h1s = [], []
                    for si in range(2):
                        i = 2 * half + si
                        g1 = g1_pool.tile([128, SUPER], f32)
                        rhs = xt_t[32 * i:32 * i + 32, u * SUPER:(u + 1) * SUPER]
                        for q in range(SUPER // 512):
                            nc.tensor.matmul(
                                out=g1[:, q * 512:(q + 1) * 512],
                                lhsT=w0p4[32 * i:32 * i + 32, :],
                                rhs=rhs[:, q * 512:(q + 1) * 512],
                                start=True, stop=True,
                                tile_position=(32 * i, 0))
                        g1s.append(g1)
                    for si in range(2):
                        h1 = h1_pool.tile([128, SUPER], bf16)
                        if n_evac % 3 < relu_act_ratio:
                            nc.scalar.activation(out=h1[:], in_=g1s[si][:], func=AF.Relu)
                        else:
                            nc.vector.tensor_scalar_max(out=h1[:], in0=g1s[si][:], scalar1=0.0)
                        n_evac += 1
                        h1s.append(h1)
                    for si in range(2):
                        i = 2 * half + si
                        g2 = g2_pool.tile([128, SUPER], f32)
                        for q in range(SUPER // 512):
                            nc.tensor.matmul(
                                out=g2[:, q * 512:(q + 1) * 512],
                                lhsT=w1bd[:, :],
                                rhs=h1s[si][:, q * 512:(q + 1) * 512],
                                start=True, stop=True)
                        phi = phi_pool.tile([128, SUPER], bf16)
                        nc.scalar.activation(out=phi[:], in_=g2[:], func=AF.Sigmoid,
                                             bias=C("b1t"))
                        seg0 = i * SEGS_PER_STRIP + (k * U_PER_DMA + u) * 64
                        nc.vector.tensor_reduce(
                            out=raw_sb[:, seg0:seg0 + 64],
                            in_=phi[:].rearrange("p (s j) -> p s j", j=PPS),
                            axis=mybir.AxisListType.X, op=ALU.add)

        p1_ctx.close()  # free phase-1 SBUF/PSUM pools
        mp_pool = ctx.enter_context(tc.tile_pool(name="mpp", bufs=2, space="PSUM"))
        vh_pool = ctx.enter_context(tc.tile_pool(name="vhp", bufs=2))

        # ---- segment means ----
        for s in range(SEGS // 512):
            sl = slice(s * 512, (s + 1) * 512)
            mp = mp_pool.tile([64, 512], f32, tag="mp64")
            nc.tensor.matmul(out=mp[:], lhsT=C("ic2"), rhs=raw_sb[:, sl],
                             start=True, stop=True)
            mt = vh_pool.tile([64, 512], f32, tag="mt")
            nc.vector.tensor_tensor(out=mt[:64], in0=mp[:], in1=cc_t[64:128, sl],
                                    op=ALU.subtract)
            nc.vector.tensor_tensor(out=means_sb[:64, sl], in0=mt[:64],
                                    in1=cc_t[0:64, sl], op=ALU.mult)

        # ---- omega MLP ----
        for s in range(VPC // 512):
            sl = slice(s * 512, (s + 1) * 512)
            p1 = mp_pool.tile([64, 512], f32, tag="mp64")
            nc.tensor.matmul(out=p1[:], lhsT=wo0p_w, rhs=xi_t[:16, sl],
                             start=True, stop=True)
            h = vh_pool.tile([64, 512], f32, tag="vh64")
            nc.scalar.activation(out=h[:64], in_=p1[:], func=AF.Relu)
            p2 = mp_pool.tile([64, 512], f32, tag="mp64")
            nc.tensor.matmul(out=p2[:], lhsT=wo1_w, rhs=h[:64], start=True, stop=True)
            nc.scalar.activation(out=omega_sb[:64, sl], in_=p2[:], func=AF.Sigmoid,
                                 bias=C("bo1", 64))

        # ---- rho + cal + epilogue, chunk-wise ----
        for s in range(VPC // 512):
            sl = slice(s * 512, (s + 1) * 512)
            rp = mp_pool.tile([128, 512], f32, tag="rp128")
            nc.tensor.matmul(out=rp[:], lhsT=wr0r_w, rhs=means_sb[:64, sl],
                             start=True, stop=False)
            nc.tensor.matmul(out=rp[:], lhsT=wr0a_w,
                             rhs=means_sb[:64, VPC + s * 512:VPC + (s + 1) * 512],
                             start=False, stop=False)
            nc.tensor.matmul(out=rp[:], lhsT=wr0o_w, rhs=omega_sb[:64, sl],
                             start=False, stop=True)
            h = vh_pool.tile([128, 512], f32, tag="vh128")
            nc.scalar.activation(out=h[:], in_=rp[:], func=AF.Relu, bias=C("br0"))
            rp2 = mp_pool.tile([64, 512], f32, tag="mp64")
            nc.tensor.matmul(out=rp2[:], lhsT=wr1_w, rhs=h[:], start=True, stop=True)
            h2 = vh_pool.tile([64, 512], f32, tag="vh64")
            nc.scalar.activation(out=h2[:64], in_=rp2[:], func=AF.Relu, bias=C("br1", 64))
            lp = mp_pool.tile([1, 512], f32, tag="mp1")
            nc.tensor.matmul(out=lp[:], lhsT=wr2_w, rhs=h2[:64], start=True, stop=True)
            lg = vh_pool.tile([1, 512], f32, tag="lg")
            nc.vector.tensor_scalar_add(out=lg[:1], in0=lp[:], scalar1=scal["br2"])

            # cal
            cp = mp_pool.tile([5, 512], f32, tag="mp5")
            nc.tensor.matmul(out=cp[:], lhsT=C("wc0", 2), rhs=sqc_t[:2, sl],
                             start=True, stop=True)
            ch = vh_pool.tile([5, 512], f32, tag="vh5")
            nc.scalar.activation(out=ch[:5], in_=cp[:], func=AF.Relu, bias=C("bc0", 5))
            cp2 = mp_pool.tile([5, 512], f32, tag="mp5")
            nc.tensor.matmul(out=cp2[:], lhsT=C("wc1", 5), rhs=ch[:5], start=True, stop=True)
            ch2 = vh_pool.tile([5, 512], f32, tag="vh5")
            nc.scalar.activation(out=ch2[:5], in_=cp2[:], func=AF.Relu, bias=C("bc1", 5))
            cp3 = mp_pool.tile([1, 512], f32, tag="mp1")
            nc.tensor.matmul(out=cp3[:], lhsT=C("wc2", 5), rhs=ch2[:5], start=True, stop=True)
            tp = vh_pool.tile([1, 512], f32, tag="tp")
            nc.vector.tensor_scalar_add(out=tp[:1], in0=cp3[:], scalar1=scal["bc2"])

            # epilogue: fin = ml * tanh(lg*tp/ml)
            ct = vh_pool.tile([1, 512], f32, tag="ct")
            nc.vector.tensor_tensor(out=ct[:1], in0=lg[:1], in1=tp[:1], op=ALU.mult)
            tt = vh_pool.tile([1, 512], f32, tag="tt")
            nc.scalar.activation(out=tt[:1], in_=ct[:1], func=AF.Tanh,
                                 scale=1.0 / scal["ml"])
            nc.vector.tensor_scalar_mul(out=fin_sb[:1, sl], in0=tt[:1],
                                        scalar1=scal["ml"])

        nc.sync.dma_start(out=out_d[:, :], in_=fin_sb[:1, :])

    nc.compile()
    return nc, CO


def _pack_consts(CO, arrs):
    blob = np.zeros((128, 704), np.float32)
    for name, a in arrs.items():
        o, w = CO[name]
        a = np.asarray(a, np.float32)
        blob[:a.shape[0], o:o + w] = a
    return blob


def kernel(**inputs):
    from concourse.bass_utils import run_bass_kernel_spmd

    reads = np.asarray(inputs["reads"], np.float32)
    info = np.asarray(inputs["info"], np.float32)
    end = np.asarray(inputs["end_indices"], np.int64)
    ml = float(np.asarray(inputs["max_logit"]))

    starts = np.concatenate([[0], end[:-1]])
    counts = (end - starts).astype(np.int64)
    assert counts.min() >= 1, "empty segments unsupported"
    PAD = max(32, 2 * math.ceil(counts.max() / 2))

    w0 = np.asarray(inputs["phi_w0"], np.float32); b0 = np.asarray(inputs["phi_b0"], np.float32)
    w1 = np.asarray(inputs["phi_w1"], np.float32); b1 = np.asarray(inputs["phi_b1"], np.float32)

    w0p = np.zeros((32, 128), np.float32)
    for a in range(2):
        w0p[16 * a:16 * a + 11, 64 * a:64 * a + 64] = w0
        w0p[16 * a + 11, 64 * a:64 * a + 64] = b0
    w0p4 = np.zeros((128, 128), np.float32)
    for i in range(NSTRIP):
        w0p4[32 * i:32 * i + 32, :] = w0p
    w1bd = np.zeros((128, 128), np.float32)
    for a in range(2):
        w1bd[64 * a:64 * a + 64, 64 * a:64 * a + 64] = w1

    def pad128(a, cols):
        out = np.zeros((128, cols), np.float32)
        a = np.asarray(a, np.float32)
        out[:a.shape[0], :a.shape[1]] = a
        return out

    wr0 = np.asarray(inputs["rho_w0"], np.float32)
    wo0p_np = np.vstack([np.asarray(inputs["omega_w0"], np.float32),
                         np.asarray(inputs["omega_b0"], np.float32)[None, :],
                         np.zeros((6, 64), np.float32)])
    wph1 = np.concatenate([w0p4, w1bd], axis=1).astype(BF16)

    v1 = (1.0 / (1.0 + np.exp(-b1))).astype(BF16).astype(np.float32)

    scal = {
        "br2": float(np.asarray(inputs["rho_b2"]).reshape(-1)[0]),
        "bc2": float(np.asarray(inputs["cal_b2"]).reshape(-1)[0]),
        "ml": ml,
    }
    nc, CO = _build_graph(PAD, scal)

    cst = _pack_consts(CO, {
        "b1t": np.tile(b1, 2)[:, None],
        "ic2": np.vstack([np.eye(64, dtype=np.float32)] * 2),
        "br0": np.asarray(inputs["rho_b0"], np.float32)[:, None],
        "br1": np.asarray(inputs["rho_b1"], np.float32)[:, None],
        "bo1": np.asarray(inputs["omega_b1"], np.float32)[:, None],
        "wc0": np.asarray(inputs["cal_w0"], np.float32),
        "wc1": np.asarray(inputs["cal_w1"], np.float32),
        "wc2": np.asarray(inputs["cal_w2"], np.float32),
        "bc0": np.asarray(inputs["cal_b0"], np.float32)[:, None],
        "bc1": np.asarray(inputs["cal_b1"], np.float32)[:, None],
        "wr0r": wr0[0:64],
        "wr0a": wr0[64:128],
        "wr0o": wr0[128:192],
        "wr1": np.asarray(inputs["rho_w1"], np.float32),
        "wr2": np.asarray(inputs["rho_w2"], np.float32),
        "wo0p": wo0p_np,
        "wo1": np.asarray(inputs["omega_w1"], np.float32),
    })

    PPS = PAD // 2
    jj = np.arange(PAD)[None, :]
    in_maps = []
    for c in range(N_CORES):
        rid = np.arange(c * VPC, (c + 1) * VPC)
        seg_ids = np.concatenate([rid, B + rid])
        cs = counts[seg_ids]
        ss = starts[seg_ids]
        valid = jj < cs[:, None]
        gidx = np.where(valid, ss[:, None] + jj, 0)
        pr = reads[gidx] * valid[:, :, None]
        feats = np.zeros((SEGS, PAD, 16), np.float32)
        feats[:, :, :11] = pr
        feats[:, :, 11] = valid
        arr = feats.reshape(NSTRIP, SEGS_PER_STRIP, PPS, 2, 16)
        xt = arr.transpose(0, 3, 4, 1, 2).reshape(128, SEGS_PER_STRIP * PPS)

        rc = (1.0 / cs).astype(np.float32)
        padn = (PAD - cs).astype(np.float32)
        cc = np.empty((128, SEGS), np.float32)
        cc[0:64] = np.broadcast_to(rc[None, :], (64, SEGS))
        cc[64:128] = v1[:, None] * padn[None, :]
        in_maps.append({
            "xt": np.ascontiguousarray(xt.astype(BF16)),
            "wph1": wph1,
            "cst": cst,
            "cc": cc,
            "sqc": np.stack([np.sqrt(cs[VPC:].astype(np.float32)),
                             np.sqrt(cs[:VPC].astype(np.float32))]),
            "xi": np.vstack([info[rid].T.astype(np.float32),
                             np.ones((1, VPC), np.float32),
                             np.zeros((6, VPC), np.float32)]),
        })

    res = run_bass_kernel_spmd(nc, in_maps, core_ids=list(range(N_CORES)))
    out = np.concatenate([res.results[c]["out"][0] for c in range(N_CORES)])
    return out.astype(np.float32)
